# revision 1
# baseline (speedup 1.0000x reference)
"""Trainium2 Bass kernel for CrossLayerSharedZOlmoeSparseMoeBlock.

Strategy (expert-parallel, 2 experts/core on 8 cores):
  K1 (device): full routing math, token-sharded 8-way -> comb [T, E] fp32
       - predictor MLP + gumbel argmax in float32r matmuls
       - router logits in full fp32 matmuls (top-k selection is
         sensitive to logit error)
       - top-8-of-16 mask via iterative max-suppression, softmax on device
  host: builds per-expert token index lists from device-computed comb
       (the "all-to-all dispatch"), gathers xT columns per expert,
       slices expert weights per core.
  K2 (device): per core, 2 experts: gate/up/down matmuls in float32r on
       compacted token buffers; gating weight applied on-chip at PSUM
       eviction. Compact outputs returned.
  host: scatter-add compact outputs into y (the "unshard/combine").

All matmuls run as float32r (TF32-like: ~1.5e-4 rel err, ~same speed as
bf16 on trn2 PE) except router logits (true fp32).
"""
import contextlib
import ctypes
import math
import os
import sys
import types

import ml_dtypes
import numpy as np

sys.path.insert(0, "/opt/trn_rl_repo")

# ---------------------------------------------------------------------------
# NTFF profile hook shim (antenv.axon_hooks is absent in this image; bass's
# trace=True path imports it). Lets us read HW exec time via neuron profile.
# ---------------------------------------------------------------------------
_SO_PATH = "/opt/axon/libaxon_pjrt.so"


def _ntff_profile_via_ctypes(so_path):
    try:
        lib = ctypes.CDLL(so_path)
    except OSError:
        return None
    if not hasattr(lib, "axon_start_nrt_profile"):
        return None
    lib.axon_start_nrt_profile.argtypes = [ctypes.POINTER(ctypes.c_int64), ctypes.c_size_t]
    lib.axon_start_nrt_profile.restype = ctypes.c_int64
    lib.axon_stop_nrt_profile.argtypes = [ctypes.c_char_p]
    lib.axon_stop_nrt_profile.restype = ctypes.c_int64

    @contextlib.contextmanager
    def _hook(output_dir, device_ids):
        import jax

        jax.devices()
        if device_ids:
            ids = (ctypes.c_int64 * len(device_ids))(*device_ids)
            rc = lib.axon_start_nrt_profile(ids, len(device_ids))
        else:
            rc = lib.axon_start_nrt_profile(None, 0)
        if rc != 0:
            raise RuntimeError(f"axon_start_nrt_profile rc={rc}")
        try:
            yield
        finally:
            n = lib.axon_stop_nrt_profile(str(output_dir).encode())
            print(f"ntff profile: {n} file(s) -> {output_dir}", file=sys.stderr)

    return _hook


def _install_hook():
    if "antenv.axon_hooks" in sys.modules:
        return
    mod = types.ModuleType("antenv.axon_hooks")
    _h = [_ntff_profile_via_ctypes(_SO_PATH)]
    mod.get_axon_ntff_profile_hook = lambda: _h[0]
    mod.set_axon_ntff_profile_hook = lambda h: _h.__setitem__(0, h)
    sys.modules["antenv.axon_hooks"] = mod
    try:
        import antenv

        antenv.axon_hooks = mod
    except ImportError:
        pass


_install_hook()

import concourse.mybir as mybir  # noqa: E402
import concourse.tile as tile  # noqa: E402
from concourse import bacc  # noqa: E402
from concourse.bass_utils import run_bass_kernel_spmd  # noqa: E402
from concourse.masks import make_identity  # noqa: E402

F32 = mybir.dt.float32
F32R = mybir.dt.float32r
BF16 = mybir.dt.bfloat16
AX = mybir.AxisListType
ALU = mybir.AluOpType
ACTF = mybir.ActivationFunctionType

# problem shapes (hardcoded per contest rules)
B, S, H = 1, 2048, 2048
T = B * S
E, F = 16, 1024
Z, M = 8, 512
TOP_K = 8
EPS = 1e-10
N_CORES = 8
E_LOC = E // N_CORES  # experts per core
TC = T // N_CORES     # tokens per core for routing
P = 128

TRACE = bool(int(os.environ.get("BASSMOE_TRACE", "0")))

_timings = {}


def r32(ap):
    return ap.bitcast(F32R)


def slice_plan(C):
    """Split C (multiple of 128) into slices <=512, multiples of 128,
    avoiding a trailing 128 (rebalance to 384+256) so f32r matmuls stay
    in the fast >=256-wide regime."""
    widths = []
    rem = C
    while rem > 0:
        w = min(512, rem)
        widths.append(w)
        rem -= w
    if len(widths) >= 2 and widths[-1] == 128:
        widths[-2:] = [384, 256]
    out, off = [], 0
    for w in widths:
        out.append((off, w))
        off += w
    return out


# ---------------------------------------------------------------------------
# K1: routing kernel (one program, token-sharded across 8 cores)
# ---------------------------------------------------------------------------
def build_k1():
    nc = bacc.Bacc(None, target_bir_lowering=False)
    xt = nc.dram_tensor("xt", [P, H // P, TC], BF16, kind="ExternalInput")
    xtf = nc.dram_tensor("xtf", [P, H // P, TC], F32, kind="ExternalInput")
    w1t = nc.dram_tensor("w1t", [M // P, P, H // P, P], BF16, kind="ExternalInput")
    w2t = nc.dram_tensor("w2t", [P, M // P, Z], BF16, kind="ExternalInput")
    gwt = nc.dram_tensor("gwt", [P, H // P, E], F32, kind="ExternalInput")
    au = nc.dram_tensor("au", [Z, E], F32, kind="ExternalInput")
    gut = nc.dram_tensor("gut", [Z, TC], F32, kind="ExternalInput")
    b1t = nc.dram_tensor("b1t", [P, M // P], F32, kind="ExternalInput")
    b2t = nc.dram_tensor("b2t", [Z, 1], F32, kind="ExternalInput")
    combo = nc.dram_tensor("combo", [TC // P, P, E], F32, kind="ExternalOutput")

    KH = H // P   # 16
    KM = M // P   # 4
    NCH = TC // P  # token chunks (2)

    with tile.TileContext(nc) as tc:
        with tc.tile_pool(name="const", bufs=1) as const, \
             tc.tile_pool(name="sb", bufs=1) as sb, \
             tc.tile_pool(name="work", bufs=2) as work, \
             tc.tile_pool(name="ps", bufs=2, space="PSUM") as ps, \
             tc.tile_pool(name="pst", bufs=1, space="PSUM") as pst:
            ident = const.tile([P, P], F32, name="ident")
            make_identity(nc, ident)
            epsc = const.tile([P, 1], F32, name="epsc")
            nc.gpsimd.memset(epsc[:], EPS)

            warm = work.tile([P, 256], BF16, name="warm")
            nc.vector.memset(warm[:], 0.0)
            for _ in range(16):
                wps = ps.tile([P, TC], F32, name="ph")
                nc.tensor.matmul(out=wps[:, :256], lhsT=warm[:, :P], rhs=warm[:],
                                 start=True, stop=True)

            xt_sb = sb.tile([P, KH, TC], BF16, name="xt_sb")
            nc.sync.dma_start(out=xt_sb[:, :KH // 2], in_=xt[:, :KH // 2])
            nc.gpsimd.dma_start(out=xt_sb[:, KH // 2:], in_=xt[:, KH // 2:])
            w1t_sb = sb.tile([P, M // P, KH, P], BF16, name="w1t_sb")
            for mm_ in range(M // P):
                nc.scalar.dma_start(out=w1t_sb[:, mm_], in_=w1t[mm_])
            xtf_sb = sb.tile([P, KH, TC], F32, name="xtf_sb")
            nc.sync.dma_start(out=xtf_sb[:], in_=xtf[:])
            w2t_sb = sb.tile([P, KM, Z], BF16, name="w2t_sb")
            nc.scalar.dma_start(out=w2t_sb[:], in_=w2t[:])
            gwt_sb = sb.tile([P, KH, E], F32, name="gwt_sb")
            nc.scalar.dma_start(out=gwt_sb[:], in_=gwt[:])
            au_sb = sb.tile([Z, E], F32, name="au_sb")
            nc.scalar.dma_start(out=au_sb[:], in_=au[:])
            gut_sb = sb.tile([Z, TC], F32, name="gut_sb")
            nc.scalar.dma_start(out=gut_sb[:], in_=gut[:])
            b1t_sb = sb.tile([P, M // P], F32, name="b1t_sb")
            nc.scalar.dma_start(out=b1t_sb[:], in_=b1t[:])
            b2t_sb = sb.tile([Z, 1], F32, name="b2t_sb")
            nc.scalar.dma_start(out=b2t_sb[:], in_=b2t[:])

            # gumbel first (groups ACT Ln table ops before Silu/Exp)
            gv = work.tile([Z, TC], F32, name="gv")
            nc.scalar.activation(out=gv[:], in_=gut_sb[:], func=ACTF.Ln,
                                 bias=epsc[:Z, 0:1], scale=1.0)
            gw = work.tile([Z, TC], F32, name="gw")
            nc.scalar.activation(out=gw[:], in_=gv[:], func=ACTF.Ln,
                                 bias=epsc[:Z, 0:1], scale=-1.0)

            # predictor: h1T = silu(W1 @ xT + b1)  [M, TC]
            h1t = sb.tile([P, KM, TC], BF16, name="h1t")
            for m in range(KM):
                ph = ps.tile([P, TC], F32, name="ph")
                for k in range(KH):
                    nc.tensor.matmul(
                        out=ph[:],
                        lhsT=w1t_sb[:, m, k, :],
                        rhs=xt_sb[:, k, :],
                        start=(k == 0), stop=(k == KH - 1),
                    )
                nc.scalar.activation(
                    out=h1t[:, m, :], in_=ph[:], func=ACTF.Silu,
                    bias=b1t_sb[:, m:m + 1], scale=1.0,
                )

            # zT = W2 @ h1T + b2   [Z, TC]
            pz = ps.tile([Z, TC], F32, name="pz")
            for mk in range(KM):
                nc.tensor.matmul(
                    out=pz[:], lhsT=w2t_sb[:, mk, :], rhs=h1t[:, mk, :],
                    start=(mk == 0), stop=(mk == KM - 1),
                )
            zt = work.tile([Z, TC], F32, name="zt")
            nc.scalar.activation(out=zt[:], in_=pz[:], func=ACTF.Identity,
                                 bias=b2t_sb[:, 0:1], scale=1.0)

            # sT = zT - w
            st = work.tile([Z, TC], F32, name="st")
            nc.vector.tensor_tensor(out=st[:], in0=zt[:], in1=gw[:], op=ALU.subtract)

            # transpose sT -> s [tok, Z] per 128-token chunk
            s_sb = work.tile([P, NCH, Z], F32, name="s_sb")
            for c in range(NCH):
                pt = pst.tile([P, Z], F32, name="pt")
                nc.tensor.transpose(
                    out=pt[:], in_=st[:, c * P:(c + 1) * P], identity=ident[:Z, :Z])
                nc.vector.tensor_copy(out=s_sb[:, c, :], in_=pt[:])

            # onehot of argmax over Z (per token)
            rmax = work.tile([P, NCH], F32, name="rmax")
            nc.vector.tensor_reduce(out=rmax[:], in_=s_sb[:], axis=AX.X, op=ALU.max)
            onehot = work.tile([P, NCH, Z], F32, name="onehot")
            for c in range(NCH):
                nc.vector.tensor_scalar(
                    out=onehot[:, c, :], in0=s_sb[:, c, :],
                    scalar1=rmax[:, c:c + 1], scalar2=None, op0=ALU.is_equal)

            # onehotT [Z, chunk*P] for router-bias matmul
            ohT = work.tile([Z, NCH, P], F32, name="ohT")
            for c in range(NCH):
                po = pst.tile([Z, P], F32, name="po")
                nc.tensor.transpose(
                    out=po[:], in_=onehot[:, c, :], identity=ident[:P, :P])
                nc.vector.tensor_copy(out=ohT[:, c, :], in_=po[:])

            # router logits rl[tok, E] = x @ gate_w.T + onehot @ (alpha U)
            rl_all = work.tile([P, NCH, E], F32, name="rl_all")
            for c in range(NCH):
                prl = pst.tile([P, E], F32, name="prl")
                for k in range(KH):
                    nc.tensor.matmul(
                        out=prl[:],
                        lhsT=xtf_sb[:, k, c * P:(c + 1) * P],
                        rhs=gwt_sb[:, k, :],
                        start=(k == 0), stop=False,
                    )
                nc.tensor.matmul(
                    out=prl[:], lhsT=ohT[:, c, :], rhs=au_sb[:],
                    start=False, stop=True,
                )
                nc.vector.tensor_copy(out=rl_all[:, c, :], in_=prl[:])

            def bcast(t):
                return t[:, :, 0:1].to_broadcast([P, NCH, E])

            # top-8 selection via DVE max8 + match_replace
            rep = work.tile([P, NCH, E], F32, name="rep")
            for c in range(NCH):
                mx8 = work.tile([P, 8], F32, name="mx8")
                nc.vector.max(out=mx8[:], in_=rl_all[:, c, :])
                nc.vector.match_replace(out=rep[:, c, :], in_to_replace=mx8[:],
                                        in_values=rl_all[:, c, :], imm_value=-1e30)

            # softmax over E
            mxn = work.tile([P, NCH, 1], F32, name="mxn")
            nc.vector.tensor_reduce(out=mxn[:, :, 0], in_=rl_all[:], axis=AX.X,
                                    op=ALU.max, negate=True)
            ex = work.tile([P, NCH, E], F32, name="ex")
            for c in range(NCH):
                nc.scalar.activation(out=ex[:, c, :], in_=rl_all[:, c, :],
                                     func=ACTF.Exp, bias=mxn[:, c, 0:1], scale=1.0)
            sm = work.tile([P, NCH, 1], F32, name="sm")
            nc.vector.tensor_reduce(out=sm[:, :, 0], in_=ex[:], axis=AX.X,
                                    op=ALU.add)
            inv = work.tile([P, NCH, 1], F32, name="inv")
            nc.vector.reciprocal(out=inv[:], in_=sm[:])

            cmb = work.tile([P, NCH, E], F32, name="cmb")
            nc.vector.tensor_tensor(out=cmb[:], in0=rl_all[:], in1=rep[:],
                                    op=ALU.not_equal)
            nc.vector.tensor_tensor(out=cmb[:], in0=cmb[:], in1=ex[:],
                                    op=ALU.mult)
            nc.vector.tensor_tensor(out=cmb[:], in0=cmb[:], in1=bcast(inv),
                                    op=ALU.mult)
            for c in range(NCH):
                nc.sync.dma_start(out=combo[c], in_=cmb[:, c, :])
    nc.compile()
    return nc


# ---------------------------------------------------------------------------
# K2: expert kernel (expert-parallel; C tokens per expert, compile-time C)
# ---------------------------------------------------------------------------
def build_k2(C):
    CC = C // P           # token chunks of 128
    CS = slice_plan(C)    # column slices
    KH = H // P           # 16
    KF = F // P           # 8
    MF = F // P           # 8 m-chunks for gate/up
    HS = H // 512         # 4

    nc = bacc.Bacc(None, target_bir_lowering=False)
    xgt = nc.dram_tensor("xgt", [E_LOC, P * KH * C], F32R, kind="ExternalInput")
    wgt = nc.dram_tensor("wgt", [E_LOC, MF, P, KH, P], F32R, kind="ExternalInput")
    wut = nc.dram_tensor("wut", [E_LOC, MF, P, KH, P], F32R, kind="ExternalInput")
    wdt = nc.dram_tensor("wdt", [E_LOC, HS, P, KF, 512], F32R, kind="ExternalInput")
    wv = nc.dram_tensor("wv", [E_LOC, P, CC], F32, kind="ExternalInput")
    outc = nc.dram_tensor("outc", [E_LOC, CC, HS, P, 512], F32, kind="ExternalOutput")

    with tile.TileContext(nc) as tc:
        with tc.tile_pool(name="xg", bufs=1) as xg_pool, \
             tc.tile_pool(name="act", bufs=1) as act_pool, \
             tc.tile_pool(name="wgu", bufs=2) as wgu_pool, \
             tc.tile_pool(name="wd", bufs=2) as wd_pool, \
             tc.tile_pool(name="wvp", bufs=2) as wv_pool, \
             tc.tile_pool(name="tmp", bufs=3) as tmp_pool, \
             tc.tile_pool(name="ev", bufs=4) as ev_pool, \
             tc.tile_pool(name="psg", bufs=2, space="PSUM") as psg, \
             tc.tile_pool(name="psu", bufs=2, space="PSUM") as psu, \
             tc.tile_pool(name="psd", bufs=3, space="PSUM") as psd:
            # PE warmup: keep HAM at 8/8 while initial DMAs land
            warm = tmp_pool.tile([P, 512], mybir.dt.bfloat16, name="warm")
            nc.vector.memset(warm[:], 0.0)
            for _ in range(32):
                wps = psd.tile([P, 512], F32, name="pd")
                nc.tensor.matmul(out=wps[:], lhsT=warm[:, :P], rhs=warm[:],
                                 start=True, stop=True)

            wd_next = None
            for e in range(E_LOC):
                xgt_sb = xg_pool.tile([P, KH, C], F32R, name="xgt_sb")
                for si, (c0, cw) in enumerate(CS):
                    if si == 0:
                        half = P * KH * cw // 2
                        nc.sync.dma_start(
                            out=xgt_sb[:, :KH // 2, c0:c0 + cw],
                            in_=xgt[e, P * KH * c0:P * KH * c0 + half])
                        nc.gpsimd.dma_start(
                            out=xgt_sb[:, KH // 2:, c0:c0 + cw],
                            in_=xgt[e, P * KH * c0 + half:P * KH * (c0 + cw)])
                    else:
                        nc.sync.dma_start(out=xgt_sb[:, :, c0:c0 + cw],
                                          in_=xgt[e, P * KH * c0:P * KH * (c0 + cw)])
                wv_sb = wv_pool.tile([P, CC], F32, name="wv_sb")
                nc.scalar.dma_start(out=wv_sb[:], in_=wv[e])
                actT = act_pool.tile([P, KF, C], F32R, name="actT")

                # gate/up projections + silu*up, output actT [F, C]
                for m in range(MF):
                    wg_sb = wgu_pool.tile([P, KH, P], F32R, name="wg_sb")
                    nc.scalar.dma_start(out=wg_sb[:], in_=wgt[e, m])
                    wu_sb = wgu_pool.tile([P, KH, P], F32R, name="wu_sb")
                    nc.scalar.dma_start(out=wu_sb[:], in_=wut[e, m])
                    if m == 1:
                        # prefetch first down-weight slice (after first gate/up cols)
                        wd_next = wd_pool.tile([P, KF, 512], F32R, name="wd_sb")
                        nc.scalar.dma_start(out=wd_next[:], in_=wdt[e, 0])
                    for (c0, cw) in CS:
                        pg = psg.tile([P, 512], F32, name="pg")[:, :cw]
                        pu = psu.tile([P, 512], F32, name="pu")[:, :cw]
                        for k in range(KH):
                            nc.tensor.matmul(
                                out=pg[:], lhsT=wg_sb[:, k, :],
                                rhs=xgt_sb[:, k, c0:c0 + cw],
                                start=(k == 0), stop=(k == KH - 1))
                        for k in range(KH):
                            nc.tensor.matmul(
                                out=pu[:], lhsT=wu_sb[:, k, :],
                                rhs=xgt_sb[:, k, c0:c0 + cw],
                                start=(k == 0), stop=(k == KH - 1))
                        sg = tmp_pool.tile([P, 512], F32, name="sg")[:, :cw]
                        nc.scalar.activation(out=sg[:], in_=pg[:], func=ACTF.Silu,
                                             bias=0.0, scale=1.0)
                        nc.vector.tensor_tensor(
                            out=actT[:, m, c0:c0 + cw], in0=sg[:], in1=pu[:],
                            op=ALU.mult)

                # down projection, gating scale at eviction, compact out rows
                for hs in range(HS):
                    wd_sb = wd_next
                    if hs < HS - 1:
                        wd_next = wd_pool.tile([P, KF, 512], F32R, name="wd_sb")
                        nc.scalar.dma_start(out=wd_next[:], in_=wdt[e, hs + 1])
                    for cc in range(CC):
                        pd = psd.tile([P, 512], F32, name="pd")
                        for k in range(KF):
                            nc.tensor.matmul(
                                out=pd[:], lhsT=actT[:, k, cc * P:(cc + 1) * P],
                                rhs=wd_sb[:, k, :],
                                start=(k == 0), stop=(k == KF - 1))
                        ev = ev_pool.tile([P, 512], F32, name="ev")
                        nc.scalar.activation(out=ev[:], in_=pd[:], func=ACTF.Copy,
                                             bias=0.0, scale=wv_sb[:, cc:cc + 1])
                        evq = nc.sync if (cc % 2 == 0) else nc.scalar
                        evq.dma_start(out=outc[e, cc, hs], in_=ev[:])
    nc.compile()
    return nc


# ---------------------------------------------------------------------------
# host orchestration
# ---------------------------------------------------------------------------
def _il(x, p=P):
    """[R, N] -> [p, R//p, N] with row r = k*p + part."""
    r, n = x.shape
    return np.ascontiguousarray(x.reshape(r // p, p, n).transpose(1, 0, 2))


_k2_cache = {}


def kernel(hidden_states, gumbel_u, W1, b1, W2, b2, gate_w, U, alpha, Wg, Wu, Wd):
    import time as _time

    t_start = _time.time()
    x = np.asarray(hidden_states, np.float32).reshape(T, H)

    # ---- host prep for K1 ----
    # xT interleaved: [128, H/128, T] with (p, k, t) = x[t, k*128+p]
    xT_il = np.ascontiguousarray(np.asarray(x).reshape(T, H // P, P).transpose(2, 1, 0))
    w1t = np.asarray(W1, np.float32).reshape(M // P, P, H // P, P).transpose(0, 3, 2, 1)
    w1t = np.ascontiguousarray(w1t).astype(ml_dtypes.bfloat16)
    w2t = _il(np.ascontiguousarray(np.asarray(W2, np.float32).T)).astype(ml_dtypes.bfloat16)
    gwt = _il(np.ascontiguousarray(np.asarray(gate_w, np.float32).T))    # [128, 16, 16]
    au = np.ascontiguousarray(np.float32(alpha) * np.asarray(U, np.float32))
    guT = np.ascontiguousarray(np.asarray(gumbel_u, np.float32).T)       # [8, T]
    b1t = np.ascontiguousarray(np.asarray(b1, np.float32).reshape(M // P, P).T)
    b2t = np.ascontiguousarray(np.asarray(b2, np.float32).reshape(Z, 1))

    in_maps1 = []
    for c in range(N_CORES):
        sl = slice(c * TC, (c + 1) * TC)
        in_maps1.append({
            "xt": np.ascontiguousarray(xT_il[:, :, sl]).astype(ml_dtypes.bfloat16),
            "xtf": np.ascontiguousarray(xT_il[:, :, sl]),
            "w1t": w1t, "w2t": w2t, "gwt": gwt, "au": au,
            "gut": np.ascontiguousarray(guT[:, sl]),
            "b1t": b1t, "b2t": b2t,
        })

    t0 = _time.time()
    nc1 = _k2_cache.get("k1")
    if nc1 is None:
        nc1 = build_k1()
        _k2_cache["k1"] = nc1
    _timings["k1_build"] = _time.time() - t0

    t0 = _time.time()
    res1 = run_bass_kernel_spmd(nc1, in_maps1, list(range(N_CORES)), trace=TRACE)
    _timings["k1_run"] = _time.time() - t0
    if TRACE:
        _timings["k1_hw_ns"] = res1.exec_time_ns

    comb = np.concatenate(
        [res1.results[c]["combo"].reshape(TC, E) for c in range(N_CORES)], axis=0)

    # ---- host routing: index lists + dispatch ----
    t0 = _time.time()
    idxs, wvals, counts = [], [], []
    for e in range(E):
        ie = np.nonzero(comb[:, e] > 0)[0].astype(np.int64)
        idxs.append(ie)
        wvals.append(comb[ie, e].astype(np.float32))
        counts.append(len(ie))
    C = max(P, P * math.ceil(max(counts) / P))

    idx_pad = np.zeros((E, C), np.int64)
    w_pad = np.zeros((E, C), np.float32)
    for e in range(E):
        n = counts[e]
        idx_pad[e, :n] = idxs[e]
        w_pad[e, :n] = wvals[e]

    # weights, transposed+interleaved+blocked (built once; per-core slices are views)
    MF, HS = F // P, H // 512
    # [E, MF, 128(p), 16(k), 128(f)]: wgt[e,m,p,k,j] = Wg[e, m*128+j, k*128+p]
    WgT = np.asarray(Wg, np.float32).reshape(E, MF, P, H // P, P).transpose(0, 1, 4, 3, 2)
    WgT = np.ascontiguousarray(WgT)
    WuT = np.asarray(Wu, np.float32).reshape(E, MF, P, H // P, P).transpose(0, 1, 4, 3, 2)
    WuT = np.ascontiguousarray(WuT)
    # [E, HS, 128(p), 8(k), 512(h)]: wdt[e,s,p,k,j] = Wd[e, s*512+j, k*128+p]
    WdT = np.asarray(Wd, np.float32).reshape(E, HS, 512, F // P, P).transpose(0, 1, 4, 3, 2)
    WdT = np.ascontiguousarray(WdT)

    in_maps2 = []
    for c in range(N_CORES):
        es = [E_LOC * c + i for i in range(E_LOC)]
        plan = slice_plan(C)
        xg_list = []
        for e in es:
            g = xT_il[:, :, idx_pad[e]]                               # [128,16,C]
            blocks = []
            for si, (c0, cw) in enumerate(plan):
                blk = g[:, :, c0:c0 + cw]
                if si == 0:
                    blocks.append(np.ascontiguousarray(blk[:, :8, :]).ravel())
                    blocks.append(np.ascontiguousarray(blk[:, 8:, :]).ravel())
                else:
                    blocks.append(blk.ravel())
            xg_list.append(np.concatenate(blocks))
        xg = np.stack(xg_list)                                        # [2, P*KH*C]
        wvc = np.stack([np.ascontiguousarray(w_pad[e].reshape(C // P, P).T)
                        for e in es])                                  # [2,128,CC]
        in_maps2.append({
            "xgt": xg,
            "wgt": WgT[es[0]:es[-1] + 1],
            "wut": WuT[es[0]:es[-1] + 1],
            "wdt": WdT[es[0]:es[-1] + 1],
            "wv": wvc,
        })
    _timings["dispatch"] = _time.time() - t0

    t0 = _time.time()
    nc2 = _k2_cache.get(("k2", C))
    if nc2 is None:
        nc2 = build_k2(C)
        _k2_cache[("k2", C)] = nc2
    _timings["k2_build"] = _time.time() - t0

    t0 = _time.time()
    res2 = run_bass_kernel_spmd(nc2, in_maps2, list(range(N_CORES)), trace=TRACE)
    _timings["k2_run"] = _time.time() - t0
    if TRACE:
        _timings["k2_hw_ns"] = res2.exec_time_ns

    # ---- host combine (unshard) ----
    t0 = _time.time()
    y = np.zeros((T, H), np.float32)
    for e in range(E):
        c, i = divmod(e, E_LOC)
        oc = res2.results[c]["outc"][i]          # [CC, HS, 128, 512]
        oc = oc.transpose(0, 2, 1, 3).reshape(-1, H)
        n = counts[e]
        y[idxs[e]] += oc[:n]
    _timings["combine"] = _time.time() - t0
    _timings["total"] = _time.time() - t_start
    return y.reshape(B, S, H)



# revision 4
# speedup vs baseline: 1.3682x; 1.3682x over previous
"""Trainium2 Bass kernel for CrossLayerSharedZOlmoeSparseMoeBlock.

Strategy (expert-parallel, 8 cores):
  host: full routing math in fp32 numpy (predictor MLP + gumbel argmax +
        router softmax + top-8-of-16) -> comb [T, E]; per-expert token
        index lists; experts paired best-with-worst by load and assigned
        2 slots/core (slot sizes are compile-time constants = max over
        cores); token buffers gathered/compacted per slot in bf16.
  device (one kernel launch): per core, per slot: gate/up/down matmuls
        in bf16 (fp32 PSUM accumulate), silu*up fused at PSUM eviction,
        routing weight applied on-chip at down-proj eviction. Exact token
        counts (no 128-padding of the moving dim).
  host: scatter-add compact fp32 outputs into y.

bf16 matmuls run at 1 cycle/row on the PE (vs ~1.8 for f32r's
fp32_mode=HIGH lowering) and halve weight/activation DMA traffic.
Aggregate rel err ~4e-3 (tolerance 2e-2).
"""
import contextlib
import ctypes
import math
import os
import sys
import types

import ml_dtypes
import numpy as np

sys.path.insert(0, "/opt/trn_rl_repo")

# ---------------------------------------------------------------------------
# NTFF profile hook shim (antenv.axon_hooks is absent in this image; bass's
# trace=True path imports it). Lets us read HW exec time via neuron profile.
# ---------------------------------------------------------------------------
_SO_PATH = "/opt/axon/libaxon_pjrt.so"


def _ntff_profile_via_ctypes(so_path):
    try:
        lib = ctypes.CDLL(so_path)
    except OSError:
        return None
    if not hasattr(lib, "axon_start_nrt_profile"):
        return None
    lib.axon_start_nrt_profile.argtypes = [ctypes.POINTER(ctypes.c_int64), ctypes.c_size_t]
    lib.axon_start_nrt_profile.restype = ctypes.c_int64
    lib.axon_stop_nrt_profile.argtypes = [ctypes.c_char_p]
    lib.axon_stop_nrt_profile.restype = ctypes.c_int64

    @contextlib.contextmanager
    def _hook(output_dir, device_ids):
        import jax

        jax.devices()
        if device_ids:
            ids = (ctypes.c_int64 * len(device_ids))(*device_ids)
            rc = lib.axon_start_nrt_profile(ids, len(device_ids))
        else:
            rc = lib.axon_start_nrt_profile(None, 0)
        if rc != 0:
            raise RuntimeError(f"axon_start_nrt_profile rc={rc}")
        try:
            yield
        finally:
            n = lib.axon_stop_nrt_profile(str(output_dir).encode())
            print(f"ntff profile: {n} file(s) -> {output_dir}", file=sys.stderr)

    return _hook


def _install_hook():
    if "antenv.axon_hooks" in sys.modules:
        return
    mod = types.ModuleType("antenv.axon_hooks")
    _h = [_ntff_profile_via_ctypes(_SO_PATH)]
    mod.get_axon_ntff_profile_hook = lambda: _h[0]
    mod.set_axon_ntff_profile_hook = lambda h: _h.__setitem__(0, h)
    sys.modules["antenv.axon_hooks"] = mod
    try:
        import antenv

        antenv.axon_hooks = mod
    except ImportError:
        pass


_install_hook()

import concourse.mybir as mybir  # noqa: E402
import concourse.tile as tile  # noqa: E402
from concourse import bacc  # noqa: E402
from concourse.bass_utils import run_bass_kernel_spmd  # noqa: E402

F32 = mybir.dt.float32
BF16 = mybir.dt.bfloat16
ALU = mybir.AluOpType
ACTF = mybir.ActivationFunctionType

# problem shapes (hardcoded per contest rules)
B, S, H = 1, 2048, 2048
T = B * S
E, F = 16, 1024
Z, M = 8, 512
TOP_K = 8
EPS = 1e-10
TAU = 1.0
N_CORES = 8
P = 128
KH = H // P          # 16 contraction chunks over H
MF = F // P          # 8 F tiles for gate/up
KF = F // P          # 8 contraction chunks over F
HS = H // 512        # 4 moving slices of 512 for down-proj
CAP = 1280           # max tokens per slot (SBUF budget guard)

TRACE = bool(int(os.environ.get("BASSMOE_TRACE", "0")))
BF = ml_dtypes.bfloat16

_timings = {}
_build_cache = {}


def _slices(C, w=512):
    out, off = [], 0
    while off < C:
        cw = min(w, C - off)
        out.append((off, cw))
        off += cw
    return out


# ---------------------------------------------------------------------------
# K2: expert kernel. sizes = per-slot token counts (compile-time).
# ---------------------------------------------------------------------------
def build_k2(sizes):
    nc = bacc.Bacc(None, target_bir_lowering=False)
    ins, outs = [], []
    for s, C in enumerate(sizes):
        CC = (C + P - 1) // P
        ins.append((
            nc.dram_tensor(f"xg{s}", [P, KH, C], BF16, kind="ExternalInput"),
            nc.dram_tensor(f"wg{s}", [MF, P, KH, P], BF16, kind="ExternalInput"),
            nc.dram_tensor(f"wu{s}", [MF, P, KH, P], BF16, kind="ExternalInput"),
            nc.dram_tensor(f"wd{s}", [HS, P, KF, 512], BF16, kind="ExternalInput"),
            nc.dram_tensor(f"wv{s}", [P, CC], F32, kind="ExternalInput"),
        ))
        outs.append(nc.dram_tensor(f"out{s}", [CC, HS, P, 512], F32,
                                   kind="ExternalOutput"))

    with tile.TileContext(nc) as tc:
        with tc.tile_pool(name="xg", bufs=2) as xg_pool, \
             tc.tile_pool(name="act", bufs=2) as act_pool, \
             tc.tile_pool(name="wgu", bufs=2) as wgu_pool, \
             tc.tile_pool(name="wd", bufs=2) as wd_pool, \
             tc.tile_pool(name="wvp", bufs=2) as wv_pool, \
             tc.tile_pool(name="tmp", bufs=3) as tmp_pool, \
             tc.tile_pool(name="ev", bufs=4) as ev_pool, \
             tc.tile_pool(name="psg", bufs=2, space="PSUM") as psg, \
             tc.tile_pool(name="psu", bufs=2, space="PSUM") as psu, \
             tc.tile_pool(name="psd", bufs=2, space="PSUM") as psd:
            # PE warmup: unthrottle HAM while initial DMAs land
            warm = tmp_pool.tile([P, 512], BF16, name="warm")
            nc.vector.memset(warm[:], 0.0)
            for i in range(24):
                wps = (psg if i % 2 == 0 else psu).tile(
                    [P, 512], F32, name=("pg" if i % 2 == 0 else "pu"))
                nc.tensor.matmul(out=wps[:], lhsT=warm[:, :P], rhs=warm[:],
                                 start=True, stop=True)

            for s, C in enumerate(sizes):
                xgD, wgD, wuD, wdD, wvD = ins[s]
                outD = outs[s]
                CC = (C + P - 1) // P
                CS = _slices(C)

                # token buffer: split DMAs by k-half x col-slice so the first
                # matmuls can start as soon as their region lands
                xg = xg_pool.tile([P, KH, C], BF16, name="xg")
                for (c0, cw) in CS:
                    nc.sync.dma_start(out=xg[:, :KH // 2, c0:c0 + cw],
                                      in_=xgD[:, :KH // 2, c0:c0 + cw])
                    nc.sync.dma_start(out=xg[:, KH // 2:, c0:c0 + cw],
                                      in_=xgD[:, KH // 2:, c0:c0 + cw])
                wv = wv_pool.tile([P, CC], F32, name="wv")
                nc.scalar.dma_start(out=wv[:], in_=wvD[:])

                actT = act_pool.tile([P, KF, C], BF16, name="actT")
                wd_tiles = [None, None]
                for m in range(MF):
                    wg = wgu_pool.tile([P, KH, P], BF16, name="wg")
                    nc.scalar.dma_start(out=wg[:], in_=wgD[m])
                    wu = wgu_pool.tile([P, KH, P], BF16, name="wu")
                    nc.scalar.dma_start(out=wu[:], in_=wuD[m])
                    if m == MF - 4:
                        wd_tiles[0] = wd_pool.tile([P, KF, 512], BF16, name="wd")
                        nc.scalar.dma_start(out=wd_tiles[0][:], in_=wdD[0])
                    if m == MF - 2:
                        wd_tiles[1] = wd_pool.tile([P, KF, 512], BF16, name="wd")
                        nc.scalar.dma_start(out=wd_tiles[1][:], in_=wdD[1])
                    for (c0, cw) in CS:
                        pg = psg.tile([P, 512], F32, name="pg")[:, :cw]
                        pu = psu.tile([P, 512], F32, name="pu")[:, :cw]
                        for k in range(KH):
                            nc.tensor.matmul(
                                out=pg[:], lhsT=wg[:, k, :],
                                rhs=xg[:, k, c0:c0 + cw],
                                start=(k == 0), stop=(k == KH - 1))
                        for k in range(KH):
                            nc.tensor.matmul(
                                out=pu[:], lhsT=wu[:, k, :],
                                rhs=xg[:, k, c0:c0 + cw],
                                start=(k == 0), stop=(k == KH - 1))
                        sg = tmp_pool.tile([P, 512], F32, name="sg")[:, :cw]
                        nc.scalar.activation(out=sg[:], in_=pg[:], func=ACTF.Silu,
                                             bias=0.0, scale=1.0)
                        nc.vector.tensor_tensor(
                            out=actT[:, m, c0:c0 + cw], in0=sg[:], in1=pu[:],
                            op=ALU.mult)

                # down projection; routing weight applied at eviction
                wd_next = wd_tiles
                for hs in range(HS):
                    wd = wd_next[hs % 2]
                    if hs < HS - 2:
                        wd_next[hs % 2] = wd_pool.tile([P, KF, 512], BF16,
                                                       name="wd")
                        nc.scalar.dma_start(out=wd_next[hs % 2][:],
                                            in_=wdD[hs + 2])
                    for cc in range(CC):
                        rows = min(P, C - cc * P)
                        pd = psd.tile([P, 512], F32, name="pd")
                        for k in range(KF):
                            nc.tensor.matmul(
                                out=pd[:rows, :],
                                lhsT=actT[:, k, cc * P:cc * P + rows],
                                rhs=wd[:, k, :],
                                start=(k == 0), stop=(k == KF - 1))
                        ev = ev_pool.tile([P, 512], F32, name="ev")
                        nc.vector.tensor_scalar(
                            out=ev[:rows, :], in0=pd[:rows, :],
                            scalar1=wv[:rows, cc:cc + 1], scalar2=None,
                            op0=ALU.mult)
                        nc.gpsimd.dma_start(out=outD[cc, hs, :rows, :],
                                            in_=ev[:rows, :])
    nc.compile()
    return nc


# ---------------------------------------------------------------------------
# host routing (exact fp32 replication of the reference)
# ---------------------------------------------------------------------------
def _host_routing(x, gumbel_u, W1, b1, W2, b2, gate_w, U, alpha):
    h1 = x @ W1.T + b1
    h1 *= 1.0 / (1.0 + np.exp(-h1))                       # silu
    zl = h1 @ W2.T + b2
    g = -np.log(-np.log(gumbel_u + EPS) + EPS)
    s = (zl + g) / TAU
    s -= s.max(-1, keepdims=True)
    es = np.exp(s)
    soft = es / es.sum(-1, keepdims=True)
    hard = np.zeros_like(soft)
    hard[np.arange(T), soft.argmax(-1)] = 1.0
    z = (hard + soft) - soft                              # straight-through
    rl = x @ gate_w.T + np.float32(alpha) * (z @ U)
    rl -= rl.max(-1, keepdims=True)
    er = np.exp(rl)
    rw = er / er.sum(-1, keepdims=True)
    order = np.argsort(-rw, axis=1, kind="stable")[:, :TOP_K]
    topw = np.take_along_axis(rw, order, axis=1)
    return order, topw


def kernel(hidden_states, gumbel_u, W1, b1, W2, b2, gate_w, U, alpha, Wg, Wu, Wd):
    import time as _time

    t_start = _time.time()
    x = np.ascontiguousarray(np.asarray(hidden_states, np.float32).reshape(T, H))

    # ---- routing on host ----
    t0 = _time.time()
    order, topw = _host_routing(
        x, np.asarray(gumbel_u, np.float32),
        np.asarray(W1, np.float32), np.asarray(b1, np.float32),
        np.asarray(W2, np.float32), np.asarray(b2, np.float32),
        np.asarray(gate_w, np.float32), np.asarray(U, np.float32), alpha)
    idxs = [None] * E
    wvals = [None] * E
    tok = np.arange(T)
    for e in range(E):
        rows, cols = np.nonzero(order == e)
        idxs[e] = rows
        wvals[e] = topw[rows, cols].astype(np.float32)
    _timings["routing"] = _time.time() - t0

    # ---- pack pieces into 8 cores x nslots ----
    t0 = _time.time()
    pieces = []
    for e in range(E):
        c = len(idxs[e])
        nparts = max(1, math.ceil(c / CAP))
        base, rem = divmod(c, nparts)
        off = 0
        for i in range(nparts):
            ln = base + (1 if i < rem else 0)
            pieces.append((e, off, ln))
            off += ln

    def cost(ln):
        return 256 * ln + 16384 * math.ceil(ln / P)

    pieces.sort(key=lambda p: -p[2])
    loads = [0] * N_CORES
    assign = [[] for _ in range(N_CORES)]
    for pc in pieces:
        c = min(range(N_CORES), key=lambda i: loads[i])
        assign[c].append(pc)
        loads[c] += cost(pc[2])
    nslots = max(len(a) for a in assign)
    for a in assign:
        a.sort(key=lambda p: -p[2])
        while len(a) < nslots:
            a.append((0, 0, 0))                            # dummy slot
    sizes = [max(P, max(assign[c][i][2] for c in range(N_CORES)))
             for i in range(nslots)]

    # ---- weight/activation prep (bf16, transposed+interleaved) ----
    xT = np.ascontiguousarray(
        x.reshape(T, KH, P).transpose(2, 1, 0).astype(BF))   # [128, 16, T]
    WgB = np.asarray(Wg, np.float32).astype(BF)
    WuB = np.asarray(Wu, np.float32).astype(BF)
    WdB = np.asarray(Wd, np.float32).astype(BF)
    # wgt[e,m,p,k,j] = Wg[e, m*128+j, k*128+p]
    WgT = np.ascontiguousarray(
        WgB.reshape(E, MF, P, KH, P).transpose(0, 1, 4, 3, 2))
    WuT = np.ascontiguousarray(
        WuB.reshape(E, MF, P, KH, P).transpose(0, 1, 4, 3, 2))
    # wdt[e,hs,p,k,j] = Wd[e, hs*512+j, k*128+p]
    WdT = np.ascontiguousarray(
        WdB.reshape(E, HS, 512, KF, P).transpose(0, 1, 4, 3, 2))

    in_maps = []
    for c in range(N_CORES):
        m = {}
        for si in range(nslots):
            e, off, ln = assign[c][si]
            Csz = sizes[si]
            CC = (Csz + P - 1) // P
            xg = np.zeros((P, KH, Csz), BF)
            wvp = np.zeros((CC * P,), np.float32)
            if ln > 0:
                sel = idxs[e][off:off + ln]
                xg[:, :, :ln] = xT[:, :, sel]
                wvp[:ln] = wvals[e][off:off + ln]
            m[f"xg{si}"] = xg
            m[f"wg{si}"] = WgT[e]
            m[f"wu{si}"] = WuT[e]
            m[f"wd{si}"] = WdT[e]
            m[f"wv{si}"] = np.ascontiguousarray(wvp.reshape(CC, P).T)
        in_maps.append(m)
    _timings["dispatch"] = _time.time() - t0

    t0 = _time.time()
    key = tuple(sizes)
    nc2 = _build_cache.get(key)
    if nc2 is None:
        nc2 = build_k2(sizes)
        _build_cache[key] = nc2
    _timings["k2_build"] = _time.time() - t0

    t0 = _time.time()
    res2 = run_bass_kernel_spmd(nc2, in_maps, list(range(N_CORES)), trace=TRACE)
    _timings["k2_run"] = _time.time() - t0
    if TRACE:
        _timings["k2_hw_ns"] = res2.exec_time_ns

    # ---- host combine (unshard) ----
    t0 = _time.time()
    y = np.zeros((T, H), np.float32)
    for c in range(N_CORES):
        for si in range(nslots):
            e, off, ln = assign[c][si]
            if ln == 0:
                continue
            CC = (sizes[si] + P - 1) // P
            oc = res2.results[c][f"out{si}"]             # [CC, HS, 128, 512]
            oc = oc.transpose(0, 2, 1, 3).reshape(CC * P, H)
            y[idxs[e][off:off + ln]] += oc[:ln]
    _timings["combine"] = _time.time() - t0
    _timings["total"] = _time.time() - t_start
    return y.reshape(B, S, H)
